# revision 2
# baseline (speedup 1.0000x reference)
"""ArDCA forward kernel for 8 trn2 NeuronCores.

z[m,i,a] = h[i,a] + sum_{j<i} sum_b J[i,j,b,a] * X[m,j,b]

Flattening (j,b)->K and (i,a)->columns, this is a block-upper-triangular
matmul Z^T = Jmat^T @ X^T where J[i].reshape(L*Q, Q) is natively the i-th
column block of the stationary operand (no transpose of J needed).

Sharding: output-column groups (6 i-positions = 126 columns each; 43 groups)
are distributed over the 8 cores into 6 uniform slots per core (SPMD needs an
identical graph on every core; which group a slot computes is decided purely
by the host-packed per-core J/h data). Each slot is one PSUM accumulation
chain over its K tiles: matmul(psum, lhsT=J_tile(128x126), rhs=XT_tile(128x512))
in bf16 with f32 accumulation, then a DVE tensor_scalar add of h evacuates
PSUM -> SBUF and the result is DMA'd out. X^T (all 42 K-tiles) is resident in
SBUF; J streams. All DRAM buffers are host-packed partition-major so each DMA
descriptor run per partition is >=1.5KB.
"""

import math
import numpy as np
import ml_dtypes

M, L, Q = 512, 256, 21
LQ = L * Q                      # 5376 = 42*128
G = 6                           # i-positions per column group
NG = (L + G - 1) // G           # 43 groups
COLS = G * Q                    # 126 matmul output partitions
NCORES = 8
NXT = LQ // 128                 # 42 X k-tiles
CX = 6                          # X k-tiles per DMA chunk (7 chunks)
CKJ = 12                        # J k-tiles per DMA chunk
BF16 = ml_dtypes.bfloat16


def _ktiles(g: int) -> int:
    i_hi = min(G * g + G, L)
    return max(1, math.ceil(Q * (i_hi - 1) / 128))


def _plan():
    """Uniform slot structure + serpentine group->core assignment."""
    items = sorted(range(NG), key=lambda g: (-_ktiles(g), g))
    nslots = math.ceil(NG / NCORES)                      # 6
    budgets = [_ktiles(items[NCORES * r]) for r in range(nslots)]
    assign = [[None] * nslots for _ in range(NCORES)]    # assign[core][slot] = group
    for r in range(nslots):
        row = items[NCORES * r: NCORES * (r + 1)]
        for k, g in enumerate(row):
            core = k if r % 2 == 0 else NCORES - 1 - k
            assign[core][r] = g
    offs = [COLS * sum(budgets[:r]) for r in range(nslots)]  # jp col offset per slot
    return budgets, assign, offs


BUDGETS, ASSIGN, JOFFS = _plan()
S = len(BUDGETS)                 # 6 slots per core
WJ = COLS * sum(BUDGETS)         # jp total columns per core
WX = NXT * M                     # xt total columns (21504)


def _build_nc():
    import concourse.bacc as bacc
    import concourse.mybir as mybir
    from concourse import tile

    f32 = mybir.dt.float32
    bf16 = mybir.dt.bfloat16

    nc = bacc.Bacc(None, target_bir_lowering=False, debug=False)
    xt_ext = nc.declare_dram_parameter("xt", [128, WX], bf16, isOutput=False)
    jp_ext = nc.declare_dram_parameter("jp", [128, WJ], bf16, isOutput=False)
    hb_ext = nc.declare_dram_parameter("hb", [COLS, S], f32, isOutput=False)
    out_ext = nc.declare_dram_parameter("out", [S * COLS, M], f32, isOutput=True)

    with tile.TileContext(nc) as tc:
        with (
            tc.tile_pool(name="x", bufs=1) as xpool,
            tc.tile_pool(name="j", bufs=3) as jpool,
            tc.tile_pool(name="ps", bufs=4, space="PSUM") as ppool,
            tc.tile_pool(name="o", bufs=3) as opool,
            tc.tile_pool(name="c", bufs=1) as cpool,
        ):
            hb_t = cpool.tile([COLS, S], f32, tag="hb")
            nc.scalar.dma_start(out=hb_t[:], in_=hb_ext[:])

            xts = []
            for ci in range(NXT // CX):
                xt_t = xpool.tile([128, CX * M], bf16, tag=f"x{ci}")
                nc.scalar.dma_start(
                    out=xt_t[:], in_=xt_ext[:, ci * CX * M:(ci + 1) * CX * M]
                )
                xts.append(xt_t)

            # slots in ascending-budget order: early slots need few X tiles,
            # letting the X DMAs race ahead of PE consumption
            for r in sorted(range(S), key=lambda r: BUDGETS[r]):
                T = BUDGETS[r]
                ps = ppool.tile([COLS, M], f32, tag="ps")
                t = 0
                while t < T:
                    ck = min(CKJ, T - t)
                    jt = jpool.tile([128, ck * COLS], bf16, tag="jc")
                    c0 = JOFFS[r] + t * COLS
                    nc.sync.dma_start(out=jt[:], in_=jp_ext[:, c0:c0 + ck * COLS])
                    for tl in range(ck):
                        tt = t + tl
                        nc.tensor.matmul(
                            ps[:],
                            jt[:, tl * COLS:(tl + 1) * COLS],
                            xts[tt // CX][:, (tt % CX) * M:(tt % CX + 1) * M],
                            start=(tt == 0),
                            stop=(tt == T - 1),
                        )
                    t += ck
                ot = opool.tile([COLS, M], f32, tag="ot")
                nc.vector.tensor_scalar_add(ot[:], ps[:], hb_t[:, r:r + 1])
                nc.gpsimd.dma_start(
                    out=out_ext[r * COLS:(r + 1) * COLS, :], in_=ot[:]
                )

    nc.finalize()
    return nc


_CACHE = {}


def _get_nc():
    if "nc" not in _CACHE:
        _CACHE["nc"] = _build_nc()
    return _CACHE["nc"]


def _pack_inputs(X_oh, h_pos, J):
    """Build per-core in_maps (host-side shard + layout)."""
    XT = np.ascontiguousarray(X_oh.transpose(1, 2, 0).reshape(LQ, M))
    xt = np.ascontiguousarray(
        XT.reshape(NXT, 128, M).transpose(1, 0, 2).reshape(128, WX)
    ).astype(BF16)

    Jb = J.reshape(L, LQ, Q).astype(BF16)  # Jb[i] = i-th column block (rows jb)

    in_maps = []
    for core in range(NCORES):
        jp = np.zeros((128, WJ), dtype=BF16)
        hb = np.zeros((COLS, S), dtype=np.float32)
        for r in range(S):
            g = ASSIGN[core][r]
            if g is None:
                continue
            T = BUDGETS[r]
            blk = np.zeros((T * 128, COLS), dtype=BF16)
            i_lo, i_hi = G * g, min(G * g + G, L)
            for il, i in enumerate(range(i_lo, i_hi)):
                rows = Q * i            # strictly-lower mask: j < i
                blk[:rows, il * Q:(il + 1) * Q] = Jb[i][:rows]
                hb[il * Q:(il + 1) * Q, r] = h_pos[i]
            jp[:, JOFFS[r]:JOFFS[r] + T * COLS] = (
                blk.reshape(T, 128, COLS).transpose(1, 0, 2).reshape(128, T * COLS)
            )
        in_maps.append({"xt": xt, "jp": jp, "hb": hb})
    return in_maps


def _unpack_outputs(results):
    outT = np.zeros((LQ, M), dtype=np.float32)
    for core in range(NCORES):
        o = results[core]["out"]
        for r in range(S):
            g = ASSIGN[core][r]
            if g is None:
                continue
            i_lo, i_hi = G * g, min(G * g + G, L)
            ncols = Q * (i_hi - i_lo)
            outT[Q * i_lo:Q * i_lo + ncols] = o[r * COLS:r * COLS + ncols]
    return np.ascontiguousarray(outT.reshape(L, Q, M).transpose(2, 0, 1))


def _run(in_maps, trace=False, **kw):
    from concourse.bass_utils import run_bass_kernel_spmd

    nc = _get_nc()
    return run_bass_kernel_spmd(nc, in_maps, list(range(NCORES)), trace=trace, **kw)


def kernel(X_oh, h_pos, J):
    X_oh = np.asarray(X_oh, dtype=np.float32)
    h_pos = np.asarray(h_pos, dtype=np.float32)
    J = np.asarray(J, dtype=np.float32)
    in_maps = _pack_inputs(X_oh, h_pos, J)
    res = _run(in_maps)
    return _unpack_outputs(res.results)


# revision 3
# speedup vs baseline: 1.2475x; 1.2475x over previous
"""ArDCA forward kernel for 8 trn2 NeuronCores.

z[m,i,a] = h[i,a] + sum_{j<i} sum_b J[i,j,b,a] * X[m,j,b]

Flattening (j,b)->K and (i,a)->columns, this is a block-upper-triangular
matmul Z^T = Jmat^T @ X^T where J[i].reshape(L*Q, Q) is natively the i-th
column block of the stationary operand (no transpose of J needed).

Sharding: output-column groups (6 i-positions = 126 cols, padded to 128 PSUM
partitions; 43 groups) are distributed over the 8 cores into 6 uniform slots
per core (SPMD needs an identical graph on every core; which group a slot
computes is decided purely by the host-packed per-core J/h data). Each slot is
one PSUM accumulation chain over its K tiles:
matmul(psum, lhsT=J_tile(128x128) bf16, rhs=XT_tile(128x512) fp8) with f32
accumulation; a DVE tensor_scalar add of h evacuates PSUM -> SBUF; result is
DMA'd out in f32. X^T (all 42 K-tiles, fp8 since one-hot 0/1 is exact) is
resident in SBUF; J streams. All DRAM buffers are host-packed partition-major
so DMA descriptor runs per partition are >=512B.
"""

import math
import numpy as np
import ml_dtypes

M, L, Q = 512, 256, 21
LQ = L * Q                      # 5376 = 42*128
G = 6                           # i-positions per column group
NG = (L + G - 1) // G           # 43 groups
COLS = 128                      # matmul/psum columns per group (126 used + 2 pad)
GCOLS = G * Q                   # 126 real columns
NCORES = 8
NXT = LQ // 128                 # 42 X k-tiles
XCHUNKS = (1, 2, 3, 6, 10, 10, 10)   # ascending: first matmuls unblock early
CKJ = 12                        # J k-tiles per DMA chunk
BF16 = ml_dtypes.bfloat16
FP8 = ml_dtypes.float8_e4m3


def _ktiles(g: int) -> int:
    i_hi = min(G * g + G, L)
    return max(1, math.ceil(Q * (i_hi - 1) / 128))


def _plan():
    """Uniform slot structure + serpentine group->core assignment."""
    items = sorted(range(NG), key=lambda g: (-_ktiles(g), g))
    nslots = math.ceil(NG / NCORES)                      # 6
    budgets = [_ktiles(items[NCORES * r]) for r in range(nslots)]
    assign = [[None] * nslots for _ in range(NCORES)]    # assign[core][slot] = group
    for r in range(nslots):
        row = items[NCORES * r: NCORES * (r + 1)]
        for k, g in enumerate(row):
            core = k if r % 2 == 0 else NCORES - 1 - k
            assign[core][r] = g
    offs = [COLS * sum(budgets[:r]) for r in range(nslots)]  # jp col offset per slot
    return budgets, assign, offs


BUDGETS, ASSIGN, JOFFS = _plan()
S = len(BUDGETS)                 # 6 slots per core
WJ = COLS * sum(BUDGETS)         # jp total columns per core
WX = NXT * M                     # xt total columns (21504)
# emission order: smallest slot first (needs only X chunk 0), then largest ->
# smallest so the big slots run while X finishes loading and the tail is short
SLOT_ORDER = [min(range(S), key=lambda r: BUDGETS[r])] + sorted(
    [r for r in range(S) if r != min(range(S), key=lambda rr: BUDGETS[rr])],
    key=lambda r: -BUDGETS[r],
)


def _build_nc():
    import concourse.bacc as bacc
    import concourse.mybir as mybir
    from concourse import tile

    f32 = mybir.dt.float32
    bf16 = mybir.dt.bfloat16
    fp8 = mybir.dt.float8e4

    nc = bacc.Bacc(None, target_bir_lowering=False, debug=False)
    xt_ext = nc.declare_dram_parameter("xt", [128, WX], fp8, isOutput=False)
    jp_ext = nc.declare_dram_parameter("jp", [128, WJ], bf16, isOutput=False)
    hb_ext = nc.declare_dram_parameter("hb", [COLS, S], f32, isOutput=False)
    out_ext = nc.declare_dram_parameter("out", [S * COLS, M], f32, isOutput=True)

    with tile.TileContext(nc) as tc:
        with (
            tc.tile_pool(name="x", bufs=1) as xpool,
            tc.tile_pool(name="j", bufs=4) as jpool,
            tc.tile_pool(name="ps", bufs=4, space="PSUM") as ppool,
            tc.tile_pool(name="o", bufs=3) as opool,
            tc.tile_pool(name="c", bufs=1) as cpool,
        ):
            hb_t = cpool.tile([COLS, S], f32, tag="hb")
            nc.sync.dma_start(out=hb_t[:], in_=hb_ext[:])

            xts = []   # (tile, first_ktile) per chunk
            xoff = 0
            for ci, cx in enumerate(XCHUNKS):
                xt_t = xpool.tile([128, cx * M], fp8, tag=f"x{ci}")
                nc.scalar.dma_start(
                    out=xt_t[:], in_=xt_ext[:, xoff * M:(xoff + cx) * M]
                )
                for t in range(cx):
                    xts.append((xt_t, t))
                xoff += cx

            for r in SLOT_ORDER:
                T = BUDGETS[r]
                ps = ppool.tile([COLS, M], f32, tag="ps")
                t = 0
                while t < T:
                    ck = min(CKJ, T - t)
                    jt = jpool.tile([128, ck * COLS], bf16, tag="jc")
                    c0 = JOFFS[r] + t * COLS
                    nc.sync.dma_start(out=jt[:], in_=jp_ext[:, c0:c0 + ck * COLS])
                    for tl in range(ck):
                        tt = t + tl
                        xt_t, xl = xts[tt]
                        nc.tensor.matmul(
                            ps[:],
                            jt[:, tl * COLS:(tl + 1) * COLS],
                            xt_t[:, xl * M:(xl + 1) * M],
                            start=(tt == 0),
                            stop=(tt == T - 1),
                        )
                    t += ck
                ot = opool.tile([COLS, M], f32, tag="ot")
                nc.vector.tensor_scalar_add(ot[:], ps[:], hb_t[:, r:r + 1])
                nc.gpsimd.dma_start(
                    out=out_ext[r * COLS:(r + 1) * COLS, :], in_=ot[:]
                )

    nc.finalize()
    return nc


_CACHE = {}


def _get_nc():
    if "nc" not in _CACHE:
        _CACHE["nc"] = _build_nc()
    return _CACHE["nc"]


def _pack_inputs(X_oh, h_pos, J):
    """Build per-core in_maps (host-side shard + layout)."""
    XT = np.ascontiguousarray(X_oh.transpose(1, 2, 0).reshape(LQ, M))
    xt = np.ascontiguousarray(
        XT.reshape(NXT, 128, M).transpose(1, 0, 2).reshape(128, WX)
    ).astype(FP8)

    Jb = J.reshape(L, LQ, Q).astype(BF16)  # Jb[i] = i-th column block (rows jb)

    in_maps = []
    for core in range(NCORES):
        jp = np.zeros((128, WJ), dtype=BF16)
        hb = np.zeros((COLS, S), dtype=np.float32)
        for r in range(S):
            g = ASSIGN[core][r]
            if g is None:
                continue
            T = BUDGETS[r]
            blk = np.zeros((T * 128, COLS), dtype=BF16)
            i_lo, i_hi = G * g, min(G * g + G, L)
            for il, i in enumerate(range(i_lo, i_hi)):
                rows = Q * i            # strictly-lower mask: j < i
                blk[:rows, il * Q:(il + 1) * Q] = Jb[i][:rows]
                hb[il * Q:(il + 1) * Q, r] = h_pos[i]
            jp[:, JOFFS[r]:JOFFS[r] + T * COLS] = (
                blk.reshape(T, 128, COLS).transpose(1, 0, 2).reshape(128, T * COLS)
            )
        in_maps.append({"xt": xt, "jp": jp, "hb": hb})
    return in_maps


def _unpack_outputs(results):
    outT = np.zeros((LQ, M), dtype=np.float32)
    for core in range(NCORES):
        o = results[core]["out"]
        for r in range(S):
            g = ASSIGN[core][r]
            if g is None:
                continue
            i_lo, i_hi = G * g, min(G * g + G, L)
            ncols = Q * (i_hi - i_lo)
            outT[Q * i_lo:Q * i_lo + ncols] = o[r * COLS:r * COLS + ncols]
    return np.ascontiguousarray(outT.reshape(L, Q, M).transpose(2, 0, 1))


def _run(in_maps, trace=False, **kw):
    from concourse.bass_utils import run_bass_kernel_spmd

    nc = _get_nc()
    return run_bass_kernel_spmd(nc, in_maps, list(range(NCORES)), trace=trace, **kw)


def kernel(X_oh, h_pos, J):
    X_oh = np.asarray(X_oh, dtype=np.float32)
    h_pos = np.asarray(h_pos, dtype=np.float32)
    J = np.asarray(J, dtype=np.float32)
    in_maps = _pack_inputs(X_oh, h_pos, J)
    res = _run(in_maps)
    return _unpack_outputs(res.results)
